# revision 4
# baseline (speedup 1.0000x reference)
"""Causal multi-head attention on 8 Trainium2 NeuronCores.

Q,K,V [2,16,2048,128] f32 -> out = causal-softmax(QK^T/sqrt(128)) V.
batch*heads = 32 -> 4 heads/core on 8 cores, fully independent.

Per head (S=2048, D=128), engine-balanced design:
  - Scores S^T[k,q] in f16: one matmul per (k-tile 128, q-block 512),
    trapezoid-narrowed on diagonal tiles. f16 keeps score error ~5e-4.
  - Causal mask: chained fp8 DoubleRow matmul adds -240 on the strict
    upper triangle of the diagonal 128x128 block (diag8 @ tri8).
  - exp is THE wall on ACT alone (1 elem/cycle/lane): split across
    ACT (exact exp, diagonal tiles forced here — they contain masked
    scores), DVE and Pool (Schraudolph bit-trick: int16 bits =
    s*A_S + B_S, bitcast f16; only on fully-valid sub-diagonal tiles
    where bits > 0 always). Static greedy balance at build time.
  - PV: f16 matmuls W^T[k,q-subtile] x [V|1] accumulating over k-tiles
    in PSUM; column 128 accumulates softmax denominators.
  - Normalize: DVE/Pool tensor_scalar divide by the denominator column.
"""

import sys

sys.path.insert(0, "/opt/trn_rl_repo")

from contextlib import ExitStack

import numpy as np
import ml_dtypes

import concourse.bass as bass
import concourse.bacc as bacc
import concourse.mybir as mybir
import concourse.tile as tile

F32 = mybir.dt.float32
F16 = mybir.dt.float16
F8E4 = mybir.dt.float8e4
I16 = mybir.dt.int16
E4NP = ml_dtypes.float8_e4m3

B, H, S, D = 2, 16, 2048, 128
NCORES = 8
HPC = (B * H) // NCORES  # 4 heads per core
P = 128
QB = 512
NQB = S // QB            # 4
NKT = S // P             # 16
VW = 132                 # padded [V|1] row width (129 used)

SCALE = 1.0 / float(np.sqrt(128.0))
LOG2E = 1.4426950408889634
EXP_BIAS = -2.0                       # w = e^(s - 2): keeps f16 range comfy
A_S = 1024.0 * LOG2E * SCALE          # Schraudolph slope on raw scores
B_S = 1024.0 * (15.0 + EXP_BIAS * LOG2E) - 50.0
MASK = -240.0
LAG = 4

Exp = mybir.ActivationFunctionType.Exp
DR = mybir.MatmulPerfMode.DoubleRow
MULT = mybir.AluOpType.mult
ADD = mybir.AluOpType.add
DIV = mybir.AluOpType.divide


def _cost_exp(eng, wd):
    if eng == "act":
        return (wd + 222) * 0.8333
    if eng == "dve":
        return (wd + 120) * 1.0417
    return (wd / 0.6) * 0.8333 + 120.0


_COST_NRM = {"dve": 258.0, "pool": 298.0}


def _emit_core(tc: tile.TileContext, ctx: ExitStack, qt_in, kt_in, v_in,
               o_out, diag_in, tri_in):
    nc = tc.nc

    const = ctx.enter_context(tc.tile_pool(name="const", bufs=1))
    big = ctx.enter_context(tc.tile_pool(name="big", bufs=2))
    wpool = ctx.enter_context(tc.tile_pool(name="w", bufs=1))
    small = ctx.enter_context(tc.tile_pool(name="small", bufs=4))
    ps_s = ctx.enter_context(tc.tile_pool(name="ps_s", bufs=4, space=bass.MemorySpace.PSUM))
    ps_o = ctx.enter_context(tc.tile_pool(name="ps_o", bufs=4, space=bass.MemorySpace.PSUM))

    diag8 = const.tile([64, 2, P], F8E4, tag="diag8")
    tri8 = const.tile([64, 2, P], F8E4, tag="tri8")
    bias_t = const.tile([P, 1], F32, tag="bias_t")
    nc.vector.memset(bias_t[:], EXP_BIAS)

    acc = {"act": 0.0, "dve": 0.0, "pool": 0.0}

    # GPSIMD (Pool) cannot access PSUM on HW, so exp (PSUM input) and
    # normalize (PSUM input) may only run on ACT and DVE.
    def emit_exp(w_slice, ps_slice, wd, force_act=False):
        if force_act:
            eng = "act"
        else:
            eng = min(("act", "dve"), key=lambda e: acc[e] + _cost_exp(e, wd))
        acc[eng] += _cost_exp(eng, wd)
        if eng == "act":
            nc.scalar.activation(w_slice, ps_slice, Exp, bias=bias_t[:],
                                 scale=SCALE)
        else:
            nc.vector.tensor_scalar(w_slice.bitcast(I16), ps_slice, A_S, B_S,
                                    MULT, ADD)

    Copy = mybir.ActivationFunctionType.Copy

    def emit_norm(out_slice, po_t, rs):
        # DVE reciprocal of the denominator column, then multiply on the
        # engine with most slack (DVE tensor_scalar or ACT Copy-with-scale)
        nc.vector.reciprocal(rs, po_t[:, P:P + 1])
        acc["dve"] += 131.0
        cost = {"act": 291.0, "dve": 258.0}
        eng = min(("act", "dve"), key=lambda e: acc[e] + cost[e])
        acc[eng] += cost[eng]
        if eng == "act":
            nc.scalar.activation(out_slice, po_t[:, 0:P], Copy, scale=rs)
        else:
            nc.vector.tensor_scalar(out_slice, po_t[:, 0:P], rs, None, MULT)

    def alloc_tiles():
        return {
            "qt": big.tile([P, S], F16, tag="qt", name="qt"),
            "kt": big.tile([P, S], F16, tag="kt", name="kt"),
            "v16": big.tile([P, NKT, VW], F16, tag="v16", name="v16"),
            "outbuf": big.tile([P, NKT, P], F16, tag="outbuf", name="outbuf"),
        }

    def load_chunk(t, h, cq):
        # chunked so the first score matmul needs only chunk 0, ~2us in
        cs = slice(cq * QB, (cq + 1) * QB)
        nc.sync.dma_start(t["kt"][:, cs], kt_in[h][:, cs])
        nc.sync.dma_start(t["qt"][:, cs], qt_in[h][:, cs])
        nc.sync.dma_start(t["v16"][:, 4 * cq:4 * (cq + 1), :],
                          v_in[h][:, 4 * cq:4 * (cq + 1), :])

    nc.gpsimd.dma_start(diag8[:], diag_in)
    nc.gpsimd.dma_start(tri8[:], tri_in)
    tiles = alloc_tiles()
    for cq in range(NQB):
        load_chunk(tiles, 0, cq)

    for h in range(HPC):
        t = tiles
        qt, kt, v16, outbuf = t["qt"], t["kt"], t["v16"], t["outbuf"]

        for qb in range(NQB):
            # prefetch next head's tensors while this head's tail computes
            if qb == 2 and h + 1 < HPC:
                tiles = alloc_tiles()
                load_chunk(tiles, h + 1, 0)
            if qb == 3 and h + 1 < HPC:
                for cq in range(1, NQB):
                    load_chunk(tiles, h + 1, cq)
            nkt = 4 * (qb + 1)
            w = wpool.tile([P, nkt, QB], F16, tag=f"w{qb}")
            po = []
            started = [False] * 4
            for _j in range(4):
                po_t = ps_o.tile([P, VW], F32, tag="po")
                po.append(po_t)
            rs_t = small.tile([P, 4], F32, tag="rs")

            def pv(j, kkt, is_first, is_last):
                nc.tensor.matmul(po[j][:, 0:P + 1],
                                 w[:, kkt, j * P:(j + 1) * P],
                                 v16[:, kkt, 0:P + 1],
                                 start=is_first, stop=is_last)

            for kkt in range(nkt):
                r = kkt - 4 * qb
                c0 = max(r, 0) * P
                ps = ps_s.tile([P, QB], F32, tag="ps")
                nc.tensor.matmul(ps[:, c0:QB],
                                 kt[:, kkt * P:(kkt + 1) * P],
                                 qt[:, qb * QB + c0:(qb + 1) * QB],
                                 start=True, stop=(r < 0))
                if r >= 0:
                    nc.tensor.matmul(ps[:, r * P:(r + 1) * P],
                                     diag8[:], tri8[:],
                                     start=False, stop=True, perf_mode=DR)
                emit_exp(w[:, kkt, c0:QB], ps[:, c0:QB], QB - c0,
                         force_act=(r >= 0))

                # PV skewed LAG k-tiles behind the scores: PE executes its
                # stream in order, so a PV right after its own score would
                # head-of-line-block on the exp. Normalize inline as each
                # chain stops so its po slot frees early.
                def pv_group(kk):
                    for j in range(4):
                        qs = 4 * qb + j
                        if kk <= qs:
                            pv(j, kk, kk == 0, kk == qs)
                            if kk == qs:
                                emit_norm(outbuf[:, qs, :], po[j],
                                          rs_t[:, j:j + 1])

                if kkt >= LAG:
                    pv_group(kkt - LAG)

            for kk in range(max(nkt - LAG, 0), nkt):
                pv_group(kk)

            # store this q-block's rows as soon as they are normalized;
            # the very last store is split so only a 2-subtile DMA (plus its
            # fixed ~1.7us completion latency) sits after the final normalize
            o_r = o_out[h].rearrange("(t p) d -> p t d", p=P)
            if h == HPC - 1 and qb == NQB - 1:
                nc.sync.dma_start(o_r[:, 12:14, :], outbuf[:, 12:14, :])
                nc.sync.dma_start(o_r[:, 14:16, :], outbuf[:, 14:16, :])
            else:
                nc.sync.dma_start(o_r[:, 4 * qb:4 * (qb + 1), :],
                                  outbuf[:, 4 * qb:4 * (qb + 1), :])


def build_nc():
    nc = bacc.Bacc("TRN2", target_bir_lowering=False, debug=False)
    qt = nc.dram_tensor("qt", [HPC, P, S], F16, kind="ExternalInput")
    kt = nc.dram_tensor("kt", [HPC, P, S], F16, kind="ExternalInput")
    v = nc.dram_tensor("v", [HPC, P, NKT, VW], F16, kind="ExternalInput")
    diag = nc.dram_tensor("diag8", [64, 2, P], F8E4, kind="ExternalInput")
    tri = nc.dram_tensor("tri8", [64, 2, P], F8E4, kind="ExternalInput")
    o = nc.dram_tensor("o", [HPC, S, D], F16, kind="ExternalOutput")
    with tile.TileContext(nc) as tc:
        with ExitStack() as ctx:
            _emit_core(tc, ctx, qt.ap(), kt.ap(), v.ap(), o.ap(),
                       diag.ap(), tri.ap())
    nc.compile()
    return nc


def make_consts():
    diag = np.zeros((64, 2, P), dtype=E4NP)
    tri = np.zeros((64, 2, P), dtype=E4NP)
    for p in range(64):
        for i in range(2):
            k = i * 64 + p
            diag[p, i, k] = E4NP(MASK)
            tri[p, i, :] = (k > np.arange(P)).astype(E4NP)
    return diag, tri


def make_in_maps(Q, K, V):
    diag, tri = make_consts()
    Qr = np.asarray(Q, dtype=np.float32).reshape(B * H, S, D)
    Kr = np.asarray(K, dtype=np.float32).reshape(B * H, S, D)
    Vr = np.asarray(V, dtype=np.float32).reshape(B * H, S, D)

    QT = np.ascontiguousarray(Qr.transpose(0, 2, 1)).astype(np.float16)
    KT = np.ascontiguousarray(Kr.transpose(0, 2, 1)).astype(np.float16)

    v16 = np.zeros((B * H, P, NKT, VW), dtype=np.float16)
    v16[:, :, :, 0:D] = Vr.reshape(B * H, NKT, P, D).transpose(0, 2, 1, 3)
    v16[:, :, :, D] = 1.0

    in_maps = []
    for c in range(NCORES):
        sl = slice(c * HPC, (c + 1) * HPC)
        in_maps.append({
            "qt": QT[sl], "kt": KT[sl], "v": v16[sl],
            "diag8": diag, "tri8": tri,
        })
    return in_maps


_NC = None


def kernel(Q: np.ndarray, K: np.ndarray, V: np.ndarray) -> np.ndarray:
    from concourse.bass_utils import run_bass_kernel_spmd

    global _NC
    if _NC is None:
        _NC = build_nc()
    nc = _NC

    in_maps = make_in_maps(Q, K, V)
    res = run_bass_kernel_spmd(nc, in_maps, core_ids=list(range(NCORES)))
    out = np.concatenate([res.results[c]["o"] for c in range(NCORES)], axis=0)
    return out.reshape(B, H, S, D).astype(np.float32)
